# revision 1
# baseline (speedup 1.0000x reference)
"""DMPNN layer kernel for Trainium2, data-parallel over batch on 8 NeuronCores.

Math (reference):
    gate[i,j]  = (sum_b adj[b,i,j]) > 0                      [N,N], shared across batch
    hW[b,i,o]  = sum_c h[b,i,c] * Wh[o,c]                    Wh = W_w[:, :H]
    term_h     = sum_i gate[i,j] * hW[b,i,o]
    e_sum      = sum_i gate[i,j] * edge_attr[b,i,j,e]
    term_e     = sum_e e_sum[b,j,e] * We[o,e]                We = W_w[:, H:]
    count[j]   = sum_i gate[i,j]
    msg        = term_h + term_e + count[j]*W_b[o]
    msg       *= (j < num_nodes[b])
    h_new      = (h + msg) @ U_w.T + U_b

Per-core layout (feature-major "T" = [hidden_on_partitions, nodes_on_free]):
  - edge_attr streamed as [i_chunk=128, (j,e)=4096] tiles (contiguous rows),
    gated by a precomputed gate_bcast [i, j*16+e] mask (DVE), reduced over i
    by ones-vector matmuls into PSUM [8,512] -> flattened to e_sum [1,4096].
  - msgT [o=128, j=256] accumulated in one PSUM bank: 2 matmuls (term_h)
    + 1 outer product (bias) + 16 outer products (term_e, rank-1 per e).
  - xT = msgT*mask + hT; h_new chunks = xT_chunk.T @ U_wT + U_b.
  - gate computed on-device from the full adj (int8, all 32 batches on every
    core) by tree-reduction over b; no cross-core collective needed.
"""

import os
import sys

for _p in ("/opt/trn_rl_repo", "/root/.axon_site/_ro/trn_rl_repo"):
    if _p not in sys.path:
        sys.path.insert(0, _p)

import numpy as np

import concourse.bass as bass
import concourse.tile as tile
from concourse import bacc, mybir
from concourse.bass_utils import run_bass_kernel_spmd

B, N, H, E = 32, 256, 128, 16
N_CORES = 8
BL = B // N_CORES          # batches per core
NJE = N * E                # 4096
F32 = mybir.dt.float32
I8 = mybir.dt.int8


def build_nc(reps: int = 1, variant: str = "flat"):
    """variant: "flat"  - e_sum flattened to [1,4096], 16 rank-1 term_e mms
                "est"   - e_sum direct to [16,256] via strided-rhs reduce mms,
                          single k=16 term_e matmul
                "fast"  - est structure + float32r matmuls (tf32-like, 4x PE
                          rate for fp32 data) + Hadamard split DVE/GpSimd"""
    est_like = variant in ("est", "fast")
    fast = variant == "fast"
    F32R = mybir.dt.float32r
    # dtype for tensors that feed fp32r matmuls: their PRODUCER instruction
    # must write float32r (walrus verifier requires rounded inputs)
    CR = F32R if fast else F32

    def rcast(ap):
        return ap.bitcast(F32R) if fast else ap

    nc = bacc.Bacc("TRN2", target_bir_lowering=False, debug=False,
                   num_devices=N_CORES)

    d_h = nc.dram_tensor("h", [BL, N, H], F32, kind="ExternalInput")
    d_ea = nc.dram_tensor("ea", [BL, N, N, E], F32, kind="ExternalInput")
    # adj bit-packed host-side (lossless encoding): bit b of word [i, j] is
    # adj[b, i, j] != 0. The any-over-batch reduction happens on device as
    # a single word != 0 compare per element.
    d_adj = nc.dram_tensor("adjb", [N, N], mybir.dt.int32,
                           kind="ExternalInput")
    d_mask = nc.dram_tensor("mask", [BL, N], F32, kind="ExternalInput")
    d_ww = nc.dram_tensor("ww", [H, H + E], F32, kind="ExternalInput")
    d_wb = nc.dram_tensor("wb", [1, H], CR, kind="ExternalInput")
    d_uw = nc.dram_tensor("uw", [H, H], F32, kind="ExternalInput")
    d_ub = nc.dram_tensor("ub", [1, H], F32, kind="ExternalInput")
    d_ident = nc.dram_tensor("ident", [128, 128], F32, kind="ExternalInput")
    d_ones = nc.dram_tensor("ones", [128, 1], CR, kind="ExternalInput")
    d_sel8 = nc.dram_tensor("sel8", [128, 64], F32, kind="ExternalInput")
    d_sel16 = nc.dram_tensor("sel16", [128, 256], CR, kind="ExternalInput")
    d_y = nc.dram_tensor("y", [BL, N, H], F32, kind="ExternalOutput")

    with tile.TileContext(nc) as tc:
        with (
            tc.tile_pool(name="const", bufs=1) as cpool,
            tc.tile_pool(name="gatep", bufs=1) as gpool,
            tc.tile_pool(name="ea", bufs=4) as eapool,
            tc.tile_pool(name="work", bufs=2) as wpool,
            tc.tile_pool(name="ps_tr", bufs=1, space="PSUM") as ps_tr,
            tc.tile_pool(name="ps_es", bufs=2, space="PSUM") as ps_es,
            tc.tile_pool(name="ps_hw", bufs=1, space="PSUM") as ps_hw,
            tc.tile_pool(name="ps_msg", bufs=2, space="PSUM") as ps_msg,
            tc.tile_pool(name="ps_up", bufs=1, space="PSUM") as ps_up,
        ):
            # ---- constants -------------------------------------------------
            ident = cpool.tile([128, 128], F32)
            nc.sync.dma_start(ident[:], d_ident[:])
            ones = cpool.tile([128, 1], CR)
            nc.sync.dma_start(ones[:], d_ones[:])
            sel8 = cpool.tile([128, 64], F32)
            nc.sync.dma_start(sel8[:], d_sel8[:])
            sel16 = cpool.tile([128, 256], CR)
            nc.sync.dma_start(sel16[:], d_sel16[:])
            ww = cpool.tile([H, H + E], F32)
            nc.sync.dma_start(ww[:], d_ww[:])
            uw = cpool.tile([H, H], F32)
            nc.sync.dma_start(uw[:], d_uw[:])
            wb = cpool.tile([1, H], CR)
            nc.sync.dma_start(wb[:], d_wb[:])
            ub_row = cpool.tile([1, H], F32)
            nc.sync.dma_start(ub_row[:], d_ub[:])

            # transposes of the weight blocks (once)
            whT = cpool.tile([H, H], CR)       # [c, o] = Wh[o, c]
            weT = cpool.tile([E, H], CR)       # [e, o] = We[o, e]
            uwT = cpool.tile([H, H], CR)       # [c, o] = U_w[o, c]
            tr_ps = ps_tr.tile([128, 128], F32, name="tr")
            nc.tensor.transpose(tr_ps[:], ww[:, 0:H], ident[:])
            nc.scalar.copy(whT[:], tr_ps[:])
            tr_ps2 = ps_tr.tile([128, 128], F32, name="tr")
            nc.tensor.transpose(tr_ps2[:E, :], ww[:, H:H + E], ident[:])
            nc.scalar.copy(weT[:], tr_ps2[:E, :])
            if not est_like:
                # flatten weT rows to partition 0 so outer-product lhsT APs
                # have base partition 0 (PE requires base in {0, 32, 64})
                weT_f = cpool.tile([1, E * H], F32)
                for e in range(E):
                    nc.sync.dma_start(weT_f[0:1, bass.ts(e, H)],
                                      weT[e:e + 1, :])
            tr_ps3 = ps_tr.tile([128, 128], F32, name="tr")
            nc.tensor.transpose(tr_ps3[:], uw[:], ident[:])
            nc.scalar.copy(uwT[:], tr_ps3[:])

            ub_b = cpool.tile([128, H], F32)    # U_b broadcast over partitions
            nc.gpsimd.partition_broadcast(ub_b[:], ub_row[0:1, :])

            for rep in range(reps):
                # ---- gate from adj (all 32 batches, tree-reduce over b) ----
                gate = []      # per i-chunk: [128, N] f32 0/1
                gate_bc = []   # per i-chunk: [128, N*E] f32, gate[i,j] at j*16+e
                for c in range(2):
                    at = gpool.tile([128, N], mybir.dt.int32,
                                    name=f"adj_t{c}")
                    nc.sync.dma_start(at[:], d_adj[bass.ts(c, 128), :])
                    g = gpool.tile([128, N], CR, name=f"gate{c}")
                    nc.vector.tensor_scalar(g[:], at[:], 0, None,
                                            mybir.AluOpType.not_equal)
                    gb = gpool.tile([128, NJE], F32, name=f"gateb{c}")
                    gb_v = gb[:].rearrange("p (j e) -> p j e", e=E)
                    for e in range(E):
                        if fast:
                            nc.scalar.copy(gb_v[:, :, e], g[:])
                        else:
                            nc.gpsimd.tensor_copy(gb_v[:, :, e], g[:])
                    gate.append(g)
                    gate_bc.append(gb)

                # count[j] = sum_i gate[i, j]
                cnt_ps = ps_tr.tile([1, N], F32, name="tr")
                for c in range(2):
                    nc.tensor.matmul(cnt_ps[:], rcast(ones[:]),
                                     rcast(gate[c][:]),
                                     start=(c == 0), stop=(c == 1))
                cnt = cpool.tile([1, N], CR, name="cnt_sb")
                nc.scalar.copy(cnt[:], cnt_ps[:])

                for b in range(BL):
                    # ---- hT [c, i] -----------------------------------------
                    hT = wpool.tile([H, N], CR, name="hT")
                    for c in range(2):
                        hn = wpool.tile([128, H], F32, name="h_nat")
                        nc.sync.dma_start(hn[:], d_h[b, bass.ts(c, 128), :])
                        htp = ps_tr.tile([128, 128], F32, name="htp")
                        nc.tensor.transpose(htp[:], hn[:], ident[:])
                        nc.scalar.copy(hT[:, bass.ts(c, 128)], htp[:])

                    # ---- hW natural [i, o], both chunks in one psum bank ---
                    hw_ps = ps_hw.tile([128, 2 * H], F32, name="hw_ps")
                    for c in range(2):
                        nc.tensor.matmul(hw_ps[:, bass.ts(c, H)],
                                         rcast(hT[:, bass.ts(c, 128)]),
                                         rcast(whT[:]),
                                         start=True, stop=True)
                    hw = wpool.tile([128, 2 * H], CR, name="hw")
                    nc.scalar.copy(hw[:], hw_ps[:])

                    # ---- gated edge stream + i-reduction -------------------
                    if not est_like:
                        es_ps = ps_es.tile([8, 512], F32, name="es_ps")
                    else:
                        es_ps = ps_es.tile([E, N], F32, name="es_ps")
                    for c in range(2):
                        ea_t = eapool.tile([128, NJE], F32, name="ea_t")
                        nc.sync.dma_start(
                            ea_t[:],
                            d_ea[b, bass.ts(c, 128), :, :].rearrange(
                                "p j e -> p (j e)"))
                        had_eng = nc.gpsimd if (fast and c == 1) else nc.vector
                        if fast:
                            # separate f32r output tile: the verifier requires
                            # every writer of an fp32r-matmul operand to round
                            # to f32r (an in-place gating would leave the DMA
                            # as an unrounded writer of the same location)
                            gea = eapool.tile([128, NJE], F32R, name="gea")
                            had_eng.tensor_tensor(gea[:], ea_t[:],
                                                  gate_bc[c][:],
                                                  mybir.AluOpType.mult)
                        else:
                            gea = ea_t
                            had_eng.tensor_tensor(ea_t[:], ea_t[:],
                                                  gate_bc[c][:],
                                                  mybir.AluOpType.mult)
                        if not est_like:
                            for t in range(8):
                                # lhsT = sel8[:, t*8:(t+1)*8]: all-ones in
                                # column t -> row t of es_ps accumulates the
                                # i-partition sum of this 512-wide slice.
                                nc.tensor.matmul(es_ps[:, :],
                                                 sel8[:, bass.ts(t, 8)],
                                                 gea[:, bass.ts(t, 512)],
                                                 start=(c == 0 and t == 0),
                                                 stop=(c == 1 and t == 7))
                        else:
                            ea_v = gea[:].rearrange("p (j e) -> p j e", e=E)
                            for e in range(E):
                                # row e of es_ps accumulates sum_i of the
                                # stride-16 j-slice for attribute e
                                nc.tensor.matmul(es_ps[:, :],
                                                 rcast(sel16[:, bass.ts(e, E)]),
                                                 rcast(ea_v[:, :, e]),
                                                 start=(c == 0 and e == 0),
                                                 stop=(c == 1 and e == E - 1))
                    if not est_like:
                        es_sb = wpool.tile([8, 512], F32, name="es_sb")
                        nc.scalar.copy(es_sb[:], es_ps[:])
                        esf = wpool.tile([1, NJE], F32, name="esf")
                        for t in range(8):
                            nc.sync.dma_start(esf[:, bass.ts(t, 512)],
                                              es_sb[t:t + 1, :])
                        esf_v = esf[:].rearrange("p (j e) -> p j e", e=E)
                    else:
                        esT_sb = wpool.tile([E, N], CR, name="es_sb")
                        nc.scalar.copy(esT_sb[:], es_ps[:])

                    # ---- msgT [o, j] accumulation --------------------------
                    msg_ps = ps_msg.tile([H, N], F32, name="msg_ps")
                    for c in range(2):
                        nc.tensor.matmul(msg_ps[:], rcast(hw[:, bass.ts(c, H)]),
                                         rcast(gate[c][:]), start=(c == 0),
                                         stop=False)
                    nc.tensor.matmul(msg_ps[:], rcast(wb[:]), rcast(cnt[:]),
                                     start=False, stop=False)
                    if not est_like:
                        for e in range(E):
                            nc.tensor.matmul(msg_ps[:],
                                             weT_f[0:1, bass.ts(e, H)],
                                             esf_v[:, :, e], start=False,
                                             stop=(e == E - 1))
                    else:
                        nc.tensor.matmul(msg_ps[:], rcast(weT[:]),
                                         rcast(esT_sb[:]),
                                         start=False, stop=True)

                    # ---- mask + add h --------------------------------------
                    mrow = wpool.tile([1, N], F32, name="mrow")
                    nc.sync.dma_start(mrow[:], d_mask[b:b + 1, :])
                    maskb = wpool.tile([128, N], F32, name="maskb")
                    nc.gpsimd.partition_broadcast(maskb[:], mrow[0:1, :])
                    xT = wpool.tile([H, N], CR, name="xT")
                    nc.vector.tensor_tensor(xT[:], msg_ps[:], maskb[:],
                                            mybir.AluOpType.mult)
                    nc.vector.tensor_tensor(xT[:], xT[:], hT[:],
                                            mybir.AluOpType.add)

                    # ---- h_new = xT.T @ uwT + ub ---------------------------
                    up_ps = ps_up.tile([128, 2 * H], F32, name="up_ps")
                    for c in range(2):
                        nc.tensor.matmul(up_ps[:, bass.ts(c, H)],
                                         rcast(xT[:, bass.ts(c, 128)]),
                                         rcast(uwT[:]),
                                         start=True, stop=True)
                    yt = wpool.tile([128, 2 * H], F32, name="yt")
                    for c in range(2):
                        nc.vector.tensor_tensor(yt[:, bass.ts(c, H)],
                                                up_ps[:, bass.ts(c, H)],
                                                ub_b[:],
                                                mybir.AluOpType.add)
                    for c in range(2):
                        nc.sync.dma_start(d_y[b, bass.ts(c, 128), :],
                                          yt[:, bass.ts(c, H)])

    nc.compile()
    return nc


def _host_prep(h, edge_attr, adj, num_nodes):
    h = np.ascontiguousarray(np.asarray(h, dtype=np.float32))
    edge_attr = np.ascontiguousarray(np.asarray(edge_attr, dtype=np.float32))
    # bit-pack adj: word [i, j] has bit b set iff adj[b, i, j] != 0
    adjb4 = np.packbits(np.asarray(adj) != 0, axis=0, bitorder='little')
    adjb = np.ascontiguousarray(adjb4.transpose(1, 2, 0)).view(
        np.uint32)[:, :, 0].astype(np.int32)
    nn = np.asarray(num_nodes).astype(np.int64)
    mask = (np.arange(N)[None, :] < nn[:, None]).astype(np.float32)
    return h, edge_attr, adjb, mask


def kernel(h, edge_attr, adj, num_nodes, W_w, W_b, U_w, U_b):
    h, edge_attr, adjb, mask = _host_prep(h, edge_attr, adj, num_nodes)
    ww = np.ascontiguousarray(np.asarray(W_w, dtype=np.float32))
    wb = np.asarray(W_b, dtype=np.float32).reshape(1, H)
    uwm = np.ascontiguousarray(np.asarray(U_w, dtype=np.float32))
    ub = np.asarray(U_b, dtype=np.float32).reshape(1, H)
    ident = np.eye(128, dtype=np.float32)
    ones = np.ones((128, 1), dtype=np.float32)
    sel8 = np.tile(np.eye(8, dtype=np.float32).reshape(1, 64), (128, 1))

    nc = build_nc(reps=1,
                  variant=os.environ.get("KERNEL_VARIANT", "fast"))
    in_maps = []
    for core in range(N_CORES):
        sl = slice(core * BL, (core + 1) * BL)
        in_maps.append({
            "h": h[sl], "ea": edge_attr[sl], "adjb": adjb,
            "mask": mask[sl], "ww": ww, "wb": wb, "uw": uwm, "ub": ub,
            "ident": ident, "ones": ones, "sel8": sel8,
            "sel16": np.tile(np.eye(16, dtype=np.float32).reshape(1, 256),
                             (128, 1)),
        })
    res = run_bass_kernel_spmd(nc, in_maps, list(range(N_CORES)))
    out = np.empty((B, N, H), dtype=np.float32)
    for core in range(N_CORES):
        out[core * BL:(core + 1) * BL] = res.results[core]["y"]
    return out



# revision 4
# speedup vs baseline: 1.9400x; 1.9400x over previous
"""DMPNN layer kernel for Trainium2, data-parallel over batch on 8 NeuronCores.

Math (reference):
    gate[i,j]  = (sum_b adj[b,i,j]) > 0                      [N,N], shared across batch
    hW[b,i,o]  = sum_c h[b,i,c] * Wh[o,c]                    Wh = W_w[:, :H]
    term_h     = sum_i gate[i,j] * hW[b,i,o]
    e_sum      = sum_i gate[i,j] * edge_attr[b,i,j,e]
    term_e     = sum_e e_sum[b,j,e] * We[o,e]                We = W_w[:, H:]
    count[j]   = sum_i gate[i,j]
    msg        = term_h + term_e + count[j]*W_b[o]
    msg       *= (j < num_nodes[b])
    h_new      = (h + msg) @ U_w.T + U_b

Key restructuring vs a direct port:
  - The gated i-reduction of edge_attr is rewritten as
        e_sum = (sum over ALL i of ea) - corr,
    where corr[b,j,e] = sum_{i: gate[i,j]=0} ea[b,i,j,e] is computed EXACTLY
    on the host (it touches only the gate-complement entries, typically none:
    gate = any-over-32-random-batches is ~all-ones). This removes the
    elementwise gate Hadamard over the 16.8MB/core ea stream entirely —
    the device reduction becomes plain ones-vector matmuls on PE.
  - All matmul operands are bf16 (PSUM accumulation stays fp32). This halves
    the dominant HBM stream. Host casts ea once.
  - Host pre-transposes h and the weight blocks, so the device does no
    PE transposes at all.
  - term_h keeps the exact gate on device (tiny [N,N] matmul rhs).

Per-core per-batch device flow (feature-major layouts, partition dim first):
  hw[i,o]   = hT_chunk.T @ whT          (2 matmuls, PSUM)
  esT[e,j]  = sum_i ea[i,(j,e)]         (16 indicator-lhsT matmuls per
                                         i-chunk, strided rhs, PSUM accum)
  msgT[o,j] = hw.T@gate + wb^T@cnt + weT.T@esT - weT.T@corrT   (PE accum)
  xT        = msgT*maskb + hT           (DVE)
  y_chunk   = xT_chunk.T @ uwT + ub     (2 matmuls + DVE bias add)
"""

import os
import sys

for _p in ("/opt/trn_rl_repo", "/root/.axon_site/_ro/trn_rl_repo"):
    if _p not in sys.path:
        sys.path.insert(0, _p)

import numpy as np

import concourse.bass as bass
import concourse.tile as tile
from concourse import bacc, mybir
from concourse.bass_utils import run_bass_kernel_spmd

B, N, H, E = 32, 256, 128, 16
N_CORES = 8
BL = B // N_CORES          # batches per core
NJE = N * E                # 4096
F32 = mybir.dt.float32
BF16 = mybir.dt.bfloat16
F32R = mybir.dt.float32r


def build_nc(reps: int = 1, variant: str = "bf16"):
    """variant: "bf16" - ea and all matmul operands bf16 (half DMA traffic)
                "f32r" - ea and matmul operands f32r (full-precision inputs)"""
    ED = BF16 if variant == "bf16" else F32R
    ea_np_align = 2 if variant == "bf16" else 4

    nc = bacc.Bacc("TRN2", target_bir_lowering=False, debug=False,
                   num_devices=N_CORES)

    d_ea = nc.dram_tensor("ea", [BL, N, N, E], ED, kind="ExternalInput")
    d_ht = nc.dram_tensor("ht", [BL, H, N], ED, kind="ExternalInput")
    d_gate = nc.dram_tensor("gate", [N, N], ED, kind="ExternalInput")
    d_cnt = nc.dram_tensor("cnt", [1, N], ED, kind="ExternalInput")
    d_corrt = nc.dram_tensor("corrt", [BL, E, N], ED, kind="ExternalInput")
    d_mask = nc.dram_tensor("mask", [1, BL * N], F32, kind="ExternalInput")
    d_wht = nc.dram_tensor("wht", [H, H], ED, kind="ExternalInput")
    d_wet = nc.dram_tensor("wet", [E, H], ED, kind="ExternalInput")
    d_nwt = nc.dram_tensor("nwt", [E, H], ED, kind="ExternalInput")
    d_uwt = nc.dram_tensor("uwt", [H, H], ED, kind="ExternalInput")
    d_wb = nc.dram_tensor("wb", [1, H], ED, kind="ExternalInput")
    d_ub2 = nc.dram_tensor("ub2", [1, 2 * H], F32, kind="ExternalInput")
    d_sel16 = nc.dram_tensor("sel16", [128, 256], ED, kind="ExternalInput")
    d_y = nc.dram_tensor("y", [BL, N, H], F32, kind="ExternalOutput")

    with tile.TileContext(nc) as tc:
        with (
            tc.tile_pool(name="const", bufs=1) as cpool,
            tc.tile_pool(name="perrep", bufs=2) as rpool,
            tc.tile_pool(name="ea", bufs=6) as eapool,
            tc.tile_pool(name="work", bufs=2) as wpool,
            tc.tile_pool(name="ps_es", bufs=2, space="PSUM") as ps_es,
            tc.tile_pool(name="ps_hw", bufs=2, space="PSUM") as ps_hw,
            tc.tile_pool(name="ps_msg", bufs=2, space="PSUM") as ps_msg,
            tc.tile_pool(name="ps_up", bufs=2, space="PSUM") as ps_up,
        ):
            # ---- constants (once per launch) ---------------------------
            wht = cpool.tile([H, H], ED)
            nc.sync.dma_start(wht[:], d_wht[:])
            wet = cpool.tile([E, H], ED)
            nc.sync.dma_start(wet[:], d_wet[:])
            nwt = cpool.tile([E, H], ED)
            nc.sync.dma_start(nwt[:], d_nwt[:])
            uwt = cpool.tile([H, H], ED)
            nc.sync.dma_start(uwt[:], d_uwt[:])
            wb = cpool.tile([1, H], ED)
            nc.sync.dma_start(wb[:], d_wb[:])
            sel16 = cpool.tile([128, 256], ED)
            nc.sync.dma_start(sel16[:], d_sel16[:])
            ub_row = cpool.tile([1, 2 * H], F32)
            nc.sync.dma_start(ub_row[:], d_ub2[:])
            ub_b = cpool.tile([128, 2 * H], F32)
            nc.gpsimd.partition_broadcast(ub_b[:], ub_row[0:1, :])

            for rep in range(reps):
                # ---- per-rep shared loads ------------------------------
                gate_t = rpool.tile([128, 2 * N], ED, name="gate")
                nc.sync.dma_start(
                    gate_t[:].rearrange("p (c j) -> p c j", c=2),
                    d_gate[:, :].rearrange("(c p) j -> p c j", c=2))
                cnt = rpool.tile([1, N], ED, name="cnt")
                nc.sync.dma_start(cnt[:], d_cnt[:])
                ht_all = rpool.tile([H, BL * N], ED, name="ht")
                nc.sync.dma_start(
                    ht_all[:].rearrange("c (b i) -> c b i", b=BL),
                    d_ht[:, :, :].rearrange("b c i -> c b i"))
                corrt = rpool.tile([E, BL * N], ED, name="corrt")
                nc.sync.dma_start(
                    corrt[:].rearrange("e (b j) -> e b j", b=BL),
                    d_corrt[:, :, :].rearrange("b e j -> e b j"))
                maskrow = rpool.tile([1, BL * N], F32, name="maskrow")
                nc.sync.dma_start(maskrow[:], d_mask[:])

                # Software-pipelined: emit PE work so that batch b's
                # post-es stages interleave with batch b+1's es stream.
                hw = [None] * BL
                es = [None] * BL
                msg = [None] * BL

                def stage_front(b):
                    # hW natural [i, o], both i-chunks in one psum bank
                    hw_ps = ps_hw.tile([128, 2 * H], F32, name="hw_ps")
                    for c in range(2):
                        nc.tensor.matmul(
                            hw_ps[:, bass.ts(c, H)],
                            ht_all[:, b * N + c * 128:b * N + (c + 1) * 128],
                            wht[:], start=True, stop=True)
                    hw_sb = wpool.tile([128, 2 * H], ED, name="hw")
                    nc.scalar.copy(hw_sb[:], hw_ps[:])
                    hw[b] = hw_sb

                    # ungated i-reduction of the ea stream -> esT [e, j]
                    es_ps = ps_es.tile([E, N], F32, name="es_ps")
                    for c in range(2):
                        ea_t = eapool.tile([128, NJE], ED, name="ea_t")
                        nc.sync.dma_start(
                            ea_t[:],
                            d_ea[b, bass.ts(c, 128), :, :].rearrange(
                                "p j e -> p (j e)"))
                        ea_v = ea_t[:].rearrange("p (j e) -> p j e", e=E)
                        for e in range(E):
                            nc.tensor.matmul(es_ps[:, :],
                                             sel16[:, bass.ts(e, E)],
                                             ea_v[:, :, e],
                                             start=(c == 0 and e == 0),
                                             stop=(c == 1 and e == E - 1))
                    esr = wpool.tile([E, N], ED, name="esr")
                    nc.scalar.copy(esr[:], es_ps[:])
                    es[b] = esr

                def stage_msg(b):
                    # msgT [o, j] accumulation in one psum bank
                    msg_ps = ps_msg.tile([H, N], F32, name="msg_ps")
                    for c in range(2):
                        nc.tensor.matmul(msg_ps[:], hw[b][:, bass.ts(c, H)],
                                         gate_t[:, bass.ts(c, N)],
                                         start=(c == 0), stop=False)
                    nc.tensor.matmul(msg_ps[:], wb[:], cnt[:],
                                     start=False, stop=False)
                    nc.tensor.matmul(msg_ps[:], wet[:], es[b][:],
                                     start=False, stop=False)
                    nc.tensor.matmul(msg_ps[:], nwt[:],
                                     corrt[:, bass.ts(b, N)],
                                     start=False, stop=True)
                    msg[b] = msg_ps

                def stage_back(b):
                    # xT = msgT*mask + hT ; y = xT_chunk.T @ uwT + ub
                    maskb = wpool.tile([128, N], F32, name="maskb")
                    nc.gpsimd.partition_broadcast(
                        maskb[:], maskrow[0:1, bass.ts(b, N)])
                    xT = wpool.tile([H, N], ED, name="xT")
                    nc.vector.tensor_tensor(xT[:], msg[b][:], maskb[:],
                                            mybir.AluOpType.mult)
                    nc.vector.tensor_tensor(xT[:], xT[:],
                                            ht_all[:, bass.ts(b, N)],
                                            mybir.AluOpType.add)
                    up_ps = ps_up.tile([128, 2 * H], F32, name="up_ps")
                    for c in range(2):
                        nc.tensor.matmul(up_ps[:, bass.ts(c, H)],
                                         xT[:, bass.ts(c, 128)],
                                         uwt[:], start=True, stop=True)
                    yt = wpool.tile([128, 2 * H], F32, name="yt")
                    nc.vector.tensor_tensor(yt[:], up_ps[:], ub_b[:],
                                            mybir.AluOpType.add)
                    nc.sync.dma_start(
                        d_y[b, :, :].rearrange("(c p) o -> p c o", c=2),
                        yt[:].rearrange("p (c o) -> p c o", c=2))

                for b in range(BL):
                    stage_front(b)
                    if b >= 1:
                        stage_msg(b - 1)
                    if b >= 2:
                        stage_back(b - 2)
                stage_msg(BL - 1)
                stage_back(BL - 2)
                stage_back(BL - 1)

    nc.compile()
    return nc


def _to_ed(a, variant):
    if variant == "bf16":
        import ml_dtypes
        return np.ascontiguousarray(a.astype(ml_dtypes.bfloat16))
    return np.ascontiguousarray(a.astype(np.float32))


def prep_inputs(h, edge_attr, adj, num_nodes, W_w, W_b, U_w, U_b,
                variant="bf16"):
    """Host-side sharding + restructuring. Returns per-core input maps."""
    h = np.asarray(h, dtype=np.float32)
    edge_attr = np.asarray(edge_attr, dtype=np.float32)
    adj = np.asarray(adj)
    nn = np.asarray(num_nodes).astype(np.int64)
    W_w = np.asarray(W_w, dtype=np.float32)
    W_b = np.asarray(W_b, dtype=np.float32)
    U_w = np.asarray(U_w, dtype=np.float32)
    U_b = np.asarray(U_b, dtype=np.float32)

    gate = (adj.sum(axis=0) > 0).astype(np.float32)          # [N, N]
    cnt = gate.sum(axis=0).reshape(1, N)                     # [1, N]
    # exact gate-complement correction: corr[b,j,e] = sum_{i:gate=0} ea[b,i,j,e]
    zmask = gate == 0
    corr = np.zeros((B, N, E), dtype=np.float32)
    jcols = np.flatnonzero(zmask.any(axis=0))
    for j in jcols:
        w = zmask[:, j].astype(np.float32)                   # [N] over i
        corr[:, j, :] = np.einsum('bie,i->be', edge_attr[:, :, j, :], w)

    mask = (np.arange(N)[None, :] < nn[:, None]).astype(np.float32)  # [B, N]
    hT = np.swapaxes(h, 1, 2)                                # [B, H, N]
    corrT = np.swapaxes(corr, 1, 2)                          # [B, E, N]
    Wh = W_w[:, :H]
    We = W_w[:, H:]
    sel16 = np.tile(np.eye(16, dtype=np.float32).reshape(1, 256), (128, 1))

    consts = {
        "gate": _to_ed(gate, variant),
        "cnt": _to_ed(cnt, variant),
        "wht": _to_ed(Wh.T, variant),
        "wet": _to_ed(We.T, variant),
        "nwt": _to_ed(-We.T, variant),
        "uwt": _to_ed(U_w.T, variant),
        "wb": _to_ed(W_b.reshape(1, H), variant),
        "ub2": np.ascontiguousarray(
            np.tile(U_b.reshape(1, H), (1, 2)).astype(np.float32)),
        "sel16": _to_ed(sel16, variant),
    }
    in_maps = []
    for core in range(N_CORES):
        sl = slice(core * BL, (core + 1) * BL)
        in_maps.append({
            "ea": _to_ed(edge_attr[sl], variant),
            "ht": _to_ed(hT[sl], variant),
            "corrt": _to_ed(corrT[sl], variant),
            "mask": np.ascontiguousarray(mask[sl].reshape(1, BL * N)),
            **consts,
        })
    return in_maps


def kernel(h, edge_attr, adj, num_nodes, W_w, W_b, U_w, U_b):
    variant = os.environ.get("KERNEL_VARIANT", "bf16")
    in_maps = prep_inputs(h, edge_attr, adj, num_nodes, W_w, W_b, U_w, U_b,
                          variant=variant)
    nc = build_nc(reps=1, variant=variant)
    res = run_bass_kernel_spmd(nc, in_maps, list(range(N_CORES)))
    out = np.empty((B, N, H), dtype=np.float32)
    for core in range(N_CORES):
        out[core * BL:(core + 1) * BL] = res.results[core]["y"]
    return out


# revision 10
# speedup vs baseline: 2.1070x; 1.0861x over previous
"""DMPNN layer kernel for Trainium2, data-parallel over batch on 8 NeuronCores.

Math (reference):
    gate[i,j]  = (sum_b adj[b,i,j]) > 0                      [N,N], shared across batch
    hW[b,i,o]  = sum_c h[b,i,c] * Wh[o,c]                    Wh = W_w[:, :H]
    term_h     = sum_i gate[i,j] * hW[b,i,o]
    e_sum      = sum_i gate[i,j] * edge_attr[b,i,j,e]
    term_e     = sum_e e_sum[b,j,e] * We[o,e]                We = W_w[:, H:]
    count[j]   = sum_i gate[i,j]
    msg        = term_h + term_e + count[j]*W_b[o]
    msg       *= (j < num_nodes[b])
    h_new      = (h + msg) @ U_w.T + U_b

Key restructuring vs a direct port:
  - The gated i-reduction of edge_attr is rewritten as
        e_sum = (sum over ALL i of ea) - corr,
    where corr[b,j,e] = sum_{i: gate[i,j]=0} ea[b,i,j,e] is computed EXACTLY
    on the host (it touches only the gate-complement entries, typically none:
    gate = any-over-32-random-batches is ~all-ones). This removes the
    elementwise gate Hadamard over the 16.8MB/core ea stream entirely —
    the device reduction becomes plain indicator-lhsT matmuls on PE.
  - All matmul operands are bf16 (PSUM accumulation stays fp32). This halves
    the dominant HBM stream. Host casts ea once.
  - Host pre-transposes h and the weight blocks; no device PE transposes.
  - term_h keeps the exact gate on device (tiny [N,N] matmul rhs).
  - The bias term count[j]*W_b[o] is folded into the correction matmul:
    corrT gets an extra row -cnt and the (negated) weight lhsT an extra
    column W_b, so  msg -= nwt17.T @ corrT17  adds both -We@corr and
    +W_b*cnt in one accumulating matmul.

DMA-ring scheduling (the real bottleneck): TRN2 has two HWDGE rings (SP and
ACT sequencers) plus the GpSimd SWDGE path; each `dma_start` occupies its
ring for the full transfer plus ~2us completion latency, serialized per
ring. So: ea batches (2MB each, the 8.4MB/core stream) alternate SP/ACT;
everything small is packed into ONE aux tensor [128, 2560] per rep on the
GpSimd ring, which also takes the y stores. PSUM->SBUF copies run on DVE so
the ACT engine is free to drive its DMA ring.
"""

import os
import sys

for _p in ("/opt/trn_rl_repo", "/root/.axon_site/_ro/trn_rl_repo"):
    if _p not in sys.path:
        sys.path.insert(0, _p)

import numpy as np

import concourse.bass as bass
import concourse.tile as tile
from concourse import bacc, mybir
from concourse.bass_utils import run_bass_kernel_spmd

B, N, H, E = 32, 256, 128, 16
N_CORES = 8
BL = B // N_CORES          # batches per core
NJE = N * E                # 4096
F32 = mybir.dt.float32
BF16 = mybir.dt.bfloat16
F32R = mybir.dt.float32r

# aux packing offsets (in ED elements, per partition)
AUX_GATE = 0               # [128, 2*N]   gate, (c j) packed
AUX_HT = 2 * N             # [128, BL*N]  hT, (b i) packed
AUX_C = AUX_HT + BL * N    # [18, BL*N]   corrT17 rows 0..16, mask row 17
AUX_W = AUX_C + BL * N     # total columns


def build_nc(reps: int = 1, variant: str = "bf16"):
    """variant: "bf16" - ea and all matmul operands bf16 (half DMA traffic)
                "f32r" - ea and matmul operands f32r (full-precision inputs)"""
    ED = BF16 if variant == "bf16" else F32R

    nc = bacc.Bacc("TRN2", target_bir_lowering=False, debug=False,
                   num_devices=N_CORES)

    d_ea = nc.dram_tensor("ea", [BL, N, N, E], ED, kind="ExternalInput")
    d_aux = nc.dram_tensor("aux", [128, AUX_W], ED, kind="ExternalInput")
    d_mask = nc.dram_tensor("mask", [1, BL * N], ED, kind="ExternalInput")
    d_wht = nc.dram_tensor("wht", [H, H], ED, kind="ExternalInput")
    d_wet = nc.dram_tensor("wet", [E, H], ED, kind="ExternalInput")
    d_nwt = nc.dram_tensor("nwt", [E + 1, H], ED, kind="ExternalInput")
    d_uwt = nc.dram_tensor("uwt", [H, H], ED, kind="ExternalInput")
    d_ub2 = nc.dram_tensor("ub2", [1, 2 * H], F32, kind="ExternalInput")
    d_sel16 = nc.dram_tensor("sel16", [128, 256], ED, kind="ExternalInput")
    d_y = nc.dram_tensor("y", [BL, N, H], F32, kind="ExternalOutput")

    with tile.TileContext(nc) as tc:
        with (
            tc.tile_pool(name="const", bufs=1) as cpool,
            tc.tile_pool(name="perrep", bufs=2) as rpool,
            tc.tile_pool(name="ea", bufs=3) as eapool,
            tc.tile_pool(name="work", bufs=2) as wpool,
            tc.tile_pool(name="ps_es", bufs=2, space="PSUM") as ps_es,
            tc.tile_pool(name="ps_hw", bufs=2, space="PSUM") as ps_hw,
            tc.tile_pool(name="ps_msg", bufs=2, space="PSUM") as ps_msg,
            tc.tile_pool(name="ps_up", bufs=2, space="PSUM") as ps_up,
        ):
            # ---- constants (once per launch) ---------------------------
            wht = cpool.tile([H, H], ED)
            nc.sync.dma_start(wht[:], d_wht[:])
            wet = cpool.tile([E, H], ED)
            nc.sync.dma_start(wet[:], d_wet[:])
            nwt = cpool.tile([E + 1, H], ED)
            nc.sync.dma_start(nwt[:], d_nwt[:])
            uwt = cpool.tile([H, H], ED)
            nc.sync.dma_start(uwt[:], d_uwt[:])
            sel16 = cpool.tile([128, 256], ED)
            nc.sync.dma_start(sel16[:], d_sel16[:])
            ub_row = cpool.tile([1, 2 * H], F32)
            nc.sync.dma_start(ub_row[:], d_ub2[:])
            ub_b = cpool.tile([128, 2 * H], F32)
            nc.gpsimd.partition_broadcast(ub_b[:], ub_row[0:1, :])

            for rep in range(reps):
                # ---- per-rep shared loads (one SWDGE DMA) --------------
                aux = rpool.tile([128, AUX_W], ED, name="aux")
                nc.gpsimd.dma_start(aux[:], d_aux[:])
                gate_t = aux[:, AUX_GATE:AUX_GATE + 2 * N]
                ht_all = aux[:, AUX_HT:AUX_HT + BL * N]
                corrt = aux[0:E + 1, AUX_C:AUX_C + BL * N]
                maskb_all = rpool.tile([128, BL * N], ED, name="maskb")
                nc.gpsimd.dma_start(maskb_all[:],
                                    d_mask[0:1, :].partition_broadcast(128))

                hw = [None] * BL
                es = [None] * BL
                msg = [None] * BL

                def stage_front(b):
                    # hW natural [i, o], both i-chunks in one psum bank
                    hw_ps = ps_hw.tile([128, 2 * H], F32, name="hw_ps")
                    for c in range(2):
                        nc.tensor.matmul(
                            hw_ps[:, bass.ts(c, H)],
                            ht_all[:, b * N + c * 128:b * N + (c + 1) * 128],
                            wht[:], start=True, stop=True)
                    hw_sb = wpool.tile([128, 2 * H], ED, name="hw")
                    nc.vector.tensor_copy(hw_sb[:], hw_ps[:])
                    hw[b] = hw_sb

                    # ungated i-reduction of the ea stream -> esT [e, j]
                    ea_t = eapool.tile([128, 2 * NJE], ED, name="ea_t")
                    eng = nc.sync if b % 2 == 0 else nc.scalar
                    eng.dma_start(
                        ea_t[:].rearrange("p (c je) -> p c je", c=2),
                        d_ea[b, :, :, :].rearrange("(c p) j e -> p c (j e)",
                                                   c=2))
                    ea_v = ea_t[:].rearrange("p (c j e) -> p c j e", c=2, e=E)
                    es_ps = ps_es.tile([E, N], F32, name="es_ps")
                    for c in range(2):
                        for e in range(E):
                            nc.tensor.matmul(es_ps[:, :],
                                             sel16[:, bass.ts(e, E)],
                                             ea_v[:, c, :, e],
                                             start=(c == 0 and e == 0),
                                             stop=(c == 1 and e == E - 1))
                    esr = wpool.tile([E, N], ED, name="esr")
                    nc.vector.tensor_copy(esr[:], es_ps[:])
                    es[b] = esr

                def stage_msg(b):
                    # msgT [o, j]: term_h (gate) + term_e - corr + bias
                    msg_ps = ps_msg.tile([H, N], F32, name="msg_ps")
                    for c in range(2):
                        nc.tensor.matmul(msg_ps[:], hw[b][:, bass.ts(c, H)],
                                         gate_t[:, bass.ts(c, N)],
                                         start=(c == 0), stop=False)
                    nc.tensor.matmul(msg_ps[:], wet[:], es[b][:],
                                     start=False, stop=False)
                    nc.tensor.matmul(msg_ps[:], nwt[:],
                                     corrt[:, bass.ts(b, N)],
                                     start=False, stop=True)
                    msg[b] = msg_ps

                def stage_back(b):
                    # xT = msgT*mask + hT ; y = xT_chunk.T @ uwT + ub
                    xT = wpool.tile([H, N], ED, name="xT")
                    nc.vector.tensor_tensor(xT[:], msg[b][:],
                                            maskb_all[:, bass.ts(b, N)],
                                            mybir.AluOpType.mult)
                    nc.vector.tensor_tensor(xT[:], xT[:],
                                            ht_all[:, bass.ts(b, N)],
                                            mybir.AluOpType.add)
                    up_ps = ps_up.tile([128, 2 * H], F32, name="up_ps")
                    for c in range(2):
                        nc.tensor.matmul(up_ps[:, bass.ts(c, H)],
                                         xT[:, bass.ts(c, 128)],
                                         uwt[:], start=True, stop=True)
                    yt = wpool.tile([128, 2 * H], F32, name="yt")
                    nc.vector.tensor_tensor(yt[:], up_ps[:], ub_b[:],
                                            mybir.AluOpType.add)
                    nc.gpsimd.dma_start(
                        d_y[b, :, :].rearrange("(c p) o -> p c o", c=2),
                        yt[:].rearrange("p (c o) -> p c o", c=2))

                for b in range(BL):
                    stage_front(b)
                    if b >= 1:
                        stage_msg(b - 1)
                    if b >= 2:
                        stage_back(b - 2)
                stage_msg(BL - 1)
                stage_back(BL - 2)
                stage_back(BL - 1)

    nc.compile()
    return nc


def _to_ed(a, variant):
    if variant == "bf16":
        import ml_dtypes
        return np.ascontiguousarray(a.astype(ml_dtypes.bfloat16))
    return np.ascontiguousarray(a.astype(np.float32))


def prep_inputs(h, edge_attr, adj, num_nodes, W_w, W_b, U_w, U_b,
                variant="bf16"):
    """Host-side sharding + restructuring. Returns per-core input maps."""
    h = np.asarray(h, dtype=np.float32)
    edge_attr = np.asarray(edge_attr, dtype=np.float32)
    adj = np.asarray(adj)
    nn = np.asarray(num_nodes).astype(np.int64)
    W_w = np.asarray(W_w, dtype=np.float32)
    W_b = np.asarray(W_b, dtype=np.float32)
    U_w = np.asarray(U_w, dtype=np.float32)
    U_b = np.asarray(U_b, dtype=np.float32)

    gate = (adj.sum(axis=0) > 0).astype(np.float32)          # [N, N]
    cnt = gate.sum(axis=0)                                   # [N]
    # exact gate-complement correction: corr[b,j,e] = sum_{i:gate=0} ea[b,i,j,e]
    zmask = gate == 0
    corr = np.zeros((B, N, E), dtype=np.float32)
    for j in np.flatnonzero(zmask.any(axis=0)):
        w = zmask[:, j].astype(np.float32)                   # [N] over i
        corr[:, j, :] = np.einsum('bie,i->be', edge_attr[:, :, j, :], w)

    mask = (np.arange(N)[None, :] < nn[:, None]).astype(np.float32)  # [B, N]
    Wh = W_w[:, :H]
    We = W_w[:, H:]
    sel16 = np.tile(np.eye(16, dtype=np.float32).reshape(1, 256), (128, 1))
    nwt17 = np.concatenate([-We.T, W_b.reshape(1, H)], axis=0)  # [17, H]

    consts = {
        "wht": _to_ed(Wh.T, variant),
        "wet": _to_ed(We.T, variant),
        "nwt": _to_ed(nwt17, variant),
        "uwt": _to_ed(U_w.T, variant),
        "ub2": np.ascontiguousarray(
            np.tile(U_b.reshape(1, H), (1, 2)).astype(np.float32)),
        "sel16": _to_ed(sel16, variant),
    }
    gate_pk = gate.reshape(2, 128, N).transpose(1, 0, 2).reshape(128, 2 * N)
    in_maps = []
    for core in range(N_CORES):
        sl = slice(core * BL, (core + 1) * BL)
        aux = np.zeros((128, AUX_W), dtype=np.float32)
        aux[:, AUX_GATE:AUX_GATE + 2 * N] = gate_pk
        # hT: aux[p, AUX_HT + b*N + i] = h[b, i, p]
        aux[:, AUX_HT:AUX_HT + BL * N] = \
            h[sl].transpose(2, 0, 1).reshape(H, BL * N)
        # corrT rows 0..15, bias row 16 = -cnt, mask row 17
        aux[0:E, AUX_C:AUX_C + BL * N] = \
            corr[sl].transpose(2, 0, 1).reshape(E, BL * N)
        aux[E, AUX_C:AUX_C + BL * N] = np.tile(cnt, BL)
        in_maps.append({
            "ea": _to_ed(edge_attr[sl], variant),
            "aux": _to_ed(aux, variant),
            "mask": _to_ed(mask[sl].reshape(1, BL * N), variant),
            **consts,
        })
    return in_maps


def kernel(h, edge_attr, adj, num_nodes, W_w, W_b, U_w, U_b):
    variant = os.environ.get("KERNEL_VARIANT", "bf16")
    in_maps = prep_inputs(h, edge_attr, adj, num_nodes, W_w, W_b, U_w, U_b,
                          variant=variant)
    nc = build_nc(reps=1, variant=variant)
    res = run_bass_kernel_spmd(nc, in_maps, list(range(N_CORES)))
    out = np.empty((B, N, H), dtype=np.float32)
    for core in range(N_CORES):
        out[core * BL:(core + 1) * BL] = res.results[core]["y"]
    return out


# revision 14
# speedup vs baseline: 4.9498x; 2.3492x over previous
"""DMPNN layer kernel for Trainium2, data-parallel over batch on 8 NeuronCores.

Math (reference):
    gate[i,j]  = (sum_b adj[b,i,j]) > 0                      [N,N], shared across batch
    hW[b,i,o]  = sum_c h[b,i,c] * Wh[o,c]                    Wh = W_w[:, :H]
    term_h     = sum_i gate[i,j] * hW[b,i,o]
    e_sum      = sum_i gate[i,j] * edge_attr[b,i,j,e]
    term_e     = sum_e e_sum[b,j,e] * We[o,e]                We = W_w[:, H:]
    count[j]   = sum_i gate[i,j]
    msg        = term_h + term_e + count[j]*W_b[o]
    msg       *= (j < num_nodes[b])
    h_new      = (h + msg) @ U_w.T + U_b

Key restructuring vs a direct port:
  - The gated i-reduction of edge_attr is rewritten as
        e_sum = (sum over ALL i of ea) - corr,
    where corr[b,j,e] = sum_{i: gate[i,j]=0} ea[b,i,j,e] is computed EXACTLY
    on the host (it touches only the gate-complement entries, typically none:
    gate = any-over-32-random-batches is ~all-ones). This removes the
    elementwise gate Hadamard over the 16.8MB/core ea stream entirely —
    the device reduction becomes plain indicator-lhsT matmuls on PE.
  - All matmul operands are bf16 (PSUM accumulation stays fp32). This halves
    the dominant HBM stream. Host casts ea once.
  - Host pre-transposes h and the weight blocks; no device PE transposes.
  - term_h keeps the exact gate on device (tiny [N,N] matmul rhs).
  - The bias term count[j]*W_b[o] is folded into the correction matmul:
    corrT gets an extra row -cnt and the (negated) weight lhsT an extra
    column W_b, so  msg -= nwt17.T @ corrT17  adds both -We@corr and
    +W_b*cnt in one accumulating matmul.

DMA-ring scheduling (the real bottleneck): TRN2 has two HWDGE rings (SP and
ACT sequencers) plus the GpSimd SWDGE path; each `dma_start` occupies its
ring for the full transfer plus ~2us completion latency, serialized per
ring. So: ea batches (2MB each, the 8.4MB/core stream) alternate SP/ACT;
everything small is packed into ONE aux tensor [128, 2560] per rep on the
GpSimd ring, which also takes the y stores. PSUM->SBUF copies run on DVE so
the ACT engine is free to drive its DMA ring.
"""

import os
import sys

for _p in ("/opt/trn_rl_repo", "/root/.axon_site/_ro/trn_rl_repo"):
    if _p not in sys.path:
        sys.path.insert(0, _p)

import numpy as np

import concourse.bass as bass
import concourse.tile as tile
from concourse import bacc, mybir
from concourse.bass_utils import run_bass_kernel_spmd

B, N, H, E = 32, 256, 128, 16
N_CORES = 8
BL = B // N_CORES          # batches per core
NJE = N * E                # 4096
F32 = mybir.dt.float32
BF16 = mybir.dt.bfloat16
F32R = mybir.dt.float32r

# aux packing offsets (in ED elements, per partition)
AUX_GATE = 0               # [128, 2*N]   gate, (c j) packed
AUX_HT = 2 * N             # [128, BL*N]  hT, (b i) packed
AUX_C = AUX_HT + BL * N    # [18, BL*N]   corrT17 rows 0..16, mask row 17
AUX_W = AUX_C + BL * N     # total columns


def build_nc(reps: int = 1, variant: str = "bf16"):
    """variant: "bf16" - ea and all matmul operands bf16 (half DMA traffic)
                "f32r" - ea and matmul operands f32r (full-precision inputs)
                "nodma" - diagnostic: es matmuls read a preloaded const tile
                          (no ea stream)
                "dmaonly" - diagnostic: ea stream + trivial consumer only"""
    nodma = variant == "nodma"
    dmaonly = variant == "dmaonly"
    ED = F32R if variant == "f32r" else BF16

    nc = bacc.Bacc("TRN2", target_bir_lowering=False, debug=False,
                   num_devices=N_CORES)

    d_ea = nc.dram_tensor("ea", [BL, N, N, E], ED, kind="ExternalInput")
    d_aux = nc.dram_tensor("aux", [128, AUX_W], ED, kind="ExternalInput")
    d_mask = nc.dram_tensor("mask", [1, BL * N], ED, kind="ExternalInput")
    d_wht = nc.dram_tensor("wht", [H, H], ED, kind="ExternalInput")
    d_wet = nc.dram_tensor("wet", [E, H], ED, kind="ExternalInput")
    d_nwt = nc.dram_tensor("nwt", [E + 1, H], ED, kind="ExternalInput")
    d_uwt = nc.dram_tensor("uwt", [H, H], ED, kind="ExternalInput")
    d_ub2 = nc.dram_tensor("ub2", [1, 2 * H], F32, kind="ExternalInput")
    d_sel16 = nc.dram_tensor("sel16", [128, 256], ED, kind="ExternalInput")
    d_y = nc.dram_tensor("y", [BL, N, H], F32, kind="ExternalOutput")

    with tile.TileContext(nc) as tc:
        with (
            tc.tile_pool(name="const", bufs=1) as cpool,
            tc.tile_pool(name="perrep", bufs=2) as rpool,
            tc.tile_pool(name="ea", bufs=3) as eapool,
            tc.tile_pool(name="work", bufs=2) as wpool,
            tc.tile_pool(name="ps_es", bufs=2, space="PSUM") as ps_es,
            tc.tile_pool(name="ps_hw", bufs=2, space="PSUM") as ps_hw,
            tc.tile_pool(name="ps_msg", bufs=2, space="PSUM") as ps_msg,
            tc.tile_pool(name="ps_up", bufs=2, space="PSUM") as ps_up,
        ):
            # ---- constants (once per launch) ---------------------------
            wht = cpool.tile([H, H], ED)
            nc.sync.dma_start(wht[:], d_wht[:])
            wet = cpool.tile([E, H], ED)
            nc.sync.dma_start(wet[:], d_wet[:])
            nwt = cpool.tile([E + 1, H], ED)
            nc.sync.dma_start(nwt[:], d_nwt[:])
            uwt = cpool.tile([H, H], ED)
            nc.sync.dma_start(uwt[:], d_uwt[:])
            sel16 = cpool.tile([128, 256], ED)
            nc.sync.dma_start(sel16[:], d_sel16[:])
            ub_row = cpool.tile([1, 2 * H], F32)
            nc.sync.dma_start(ub_row[:], d_ub2[:])
            ub_b = cpool.tile([128, 2 * H], F32)
            nc.gpsimd.partition_broadcast(ub_b[:], ub_row[0:1, :])

            for rep in range(reps):
                # ---- per-rep shared loads (one SWDGE DMA) --------------
                aux = rpool.tile([128, AUX_W], ED, name="aux")
                nc.gpsimd.dma_start(aux[:], d_aux[:])
                gate_t = aux[:, AUX_GATE:AUX_GATE + 2 * N]
                ht_all = aux[:, AUX_HT:AUX_HT + BL * N]
                corrt = aux[0:E + 1, AUX_C:AUX_C + BL * N]
                maskb_all = rpool.tile([128, BL * N], ED, name="maskb")
                nc.gpsimd.dma_start(maskb_all[:],
                                    d_mask[0:1, :].partition_broadcast(128))

                hw = [None] * BL
                es = [None] * BL
                msg = [None] * BL

                def stage_front(b):
                    # hW natural [i, o], both i-chunks in one psum bank
                    hw_ps = ps_hw.tile([128, 2 * H], F32, name="hw_ps")
                    for c in range(2):
                        nc.tensor.matmul(
                            hw_ps[:, bass.ts(c, H)],
                            ht_all[:, b * N + c * 128:b * N + (c + 1) * 128],
                            wht[:], start=True, stop=True)
                    hw_sb = wpool.tile([128, 2 * H], ED, name="hw")
                    nc.vector.tensor_copy(hw_sb[:], hw_ps[:])
                    hw[b] = hw_sb

                    # ungated i-reduction of the ea stream -> esT [e, j]
                    if nodma:
                        ea_t = aux  # any preloaded [128, >=2*NJE]... reuse
                    else:
                        ea_t = eapool.tile([128, 2 * NJE], ED, name="ea_t")
                        eng = nc.sync if b % 2 == 0 else nc.scalar
                        eng.dma_start(
                            ea_t[:].rearrange("p (c je) -> p c je", c=2),
                            d_ea[b, :, :, :].rearrange(
                                "(c p) j e -> p c (j e)", c=2))
                    es_ps = ps_es.tile([E, N], F32, name="es_ps")
                    if nodma:
                        for c in range(2):
                            for e in range(E):
                                nc.tensor.matmul(
                                    es_ps[:, :], sel16[:, bass.ts(e, E)],
                                    aux[:, 0:N],
                                    start=(c == 0 and e == 0),
                                    stop=(c == 1 and e == E - 1))
                    elif dmaonly:
                        ea_v = ea_t[:].rearrange("p (c j e) -> p c j e",
                                                 c=2, e=E)
                        nc.tensor.matmul(es_ps[:, :], sel16[:, 0:E],
                                         ea_v[:, 0, :, 0],
                                         start=True, stop=True)
                    else:
                        ea_v = ea_t[:].rearrange("p (c j e) -> p c j e",
                                                 c=2, e=E)
                        for c in range(2):
                            for e in range(E):
                                nc.tensor.matmul(es_ps[:, :],
                                                 sel16[:, bass.ts(e, E)],
                                                 ea_v[:, c, :, e],
                                                 start=(c == 0 and e == 0),
                                                 stop=(c == 1 and e == E - 1))
                    esr = wpool.tile([E, N], ED, name="esr")
                    nc.vector.tensor_copy(esr[:], es_ps[:])
                    es[b] = esr

                def stage_msg(b):
                    # msgT [o, j]: term_h (gate) + term_e - corr + bias
                    msg_ps = ps_msg.tile([H, N], F32, name="msg_ps")
                    for c in range(2):
                        nc.tensor.matmul(msg_ps[:], hw[b][:, bass.ts(c, H)],
                                         gate_t[:, bass.ts(c, N)],
                                         start=(c == 0), stop=False)
                    nc.tensor.matmul(msg_ps[:], wet[:], es[b][:],
                                     start=False, stop=False)
                    nc.tensor.matmul(msg_ps[:], nwt[:],
                                     corrt[:, bass.ts(b, N)],
                                     start=False, stop=True)
                    msg[b] = msg_ps

                def stage_back(b):
                    # xT = msgT*mask + hT ; y = xT_chunk.T @ uwT + ub
                    xT = wpool.tile([H, N], ED, name="xT")
                    nc.vector.tensor_tensor(xT[:], msg[b][:],
                                            maskb_all[:, bass.ts(b, N)],
                                            mybir.AluOpType.mult)
                    nc.vector.tensor_tensor(xT[:], xT[:],
                                            ht_all[:, bass.ts(b, N)],
                                            mybir.AluOpType.add)
                    up_ps = ps_up.tile([128, 2 * H], F32, name="up_ps")
                    for c in range(2):
                        nc.tensor.matmul(up_ps[:, bass.ts(c, H)],
                                         xT[:, bass.ts(c, 128)],
                                         uwt[:], start=True, stop=True)
                    yt = wpool.tile([128, 2 * H], F32, name="yt")
                    nc.vector.tensor_tensor(yt[:], up_ps[:], ub_b[:],
                                            mybir.AluOpType.add)
                    nc.gpsimd.dma_start(
                        d_y[b, :, :].rearrange("(c p) o -> p c o", c=2),
                        yt[:].rearrange("p (c o) -> p c o", c=2))

                for b in range(BL):
                    stage_front(b)
                    if b >= 1:
                        stage_msg(b - 1)
                    if b >= 2:
                        stage_back(b - 2)
                stage_msg(BL - 1)
                stage_back(BL - 2)
                stage_back(BL - 1)

    nc.compile()
    return nc


def _to_ed(a, variant):
    if variant == "f32r":
        return np.ascontiguousarray(a.astype(np.float32))
    import ml_dtypes
    return np.ascontiguousarray(a.astype(ml_dtypes.bfloat16))


def prep_inputs(h, edge_attr, adj, num_nodes, W_w, W_b, U_w, U_b,
                variant="bf16"):
    """Host-side sharding + restructuring. Returns per-core input maps."""
    h = np.asarray(h, dtype=np.float32)
    edge_attr = np.asarray(edge_attr, dtype=np.float32)
    adj = np.asarray(adj)
    nn = np.asarray(num_nodes).astype(np.int64)
    W_w = np.asarray(W_w, dtype=np.float32)
    W_b = np.asarray(W_b, dtype=np.float32)
    U_w = np.asarray(U_w, dtype=np.float32)
    U_b = np.asarray(U_b, dtype=np.float32)

    gate = (adj.sum(axis=0) > 0).astype(np.float32)          # [N, N]
    cnt = gate.sum(axis=0)                                   # [N]
    # exact gate-complement correction: corr[b,j,e] = sum_{i:gate=0} ea[b,i,j,e]
    zmask = gate == 0
    corr = np.zeros((B, N, E), dtype=np.float32)
    for j in np.flatnonzero(zmask.any(axis=0)):
        w = zmask[:, j].astype(np.float32)                   # [N] over i
        corr[:, j, :] = np.einsum('bie,i->be', edge_attr[:, :, j, :], w)

    mask = (np.arange(N)[None, :] < nn[:, None]).astype(np.float32)  # [B, N]
    Wh = W_w[:, :H]
    We = W_w[:, H:]
    sel16 = np.tile(np.eye(16, dtype=np.float32).reshape(1, 256), (128, 1))
    nwt17 = np.concatenate([-We.T, W_b.reshape(1, H)], axis=0)  # [17, H]

    consts = {
        "wht": _to_ed(Wh.T, variant),
        "wet": _to_ed(We.T, variant),
        "nwt": _to_ed(nwt17, variant),
        "uwt": _to_ed(U_w.T, variant),
        "ub2": np.ascontiguousarray(
            np.tile(U_b.reshape(1, H), (1, 2)).astype(np.float32)),
        "sel16": _to_ed(sel16, variant),
    }
    gate_pk = gate.reshape(2, 128, N).transpose(1, 0, 2).reshape(128, 2 * N)
    in_maps = []
    for core in range(N_CORES):
        sl = slice(core * BL, (core + 1) * BL)
        aux = np.zeros((128, AUX_W), dtype=np.float32)
        aux[:, AUX_GATE:AUX_GATE + 2 * N] = gate_pk
        # hT: aux[p, AUX_HT + b*N + i] = h[b, i, p]
        aux[:, AUX_HT:AUX_HT + BL * N] = \
            h[sl].transpose(2, 0, 1).reshape(H, BL * N)
        # corrT rows 0..15, bias row 16 = -cnt, mask row 17
        aux[0:E, AUX_C:AUX_C + BL * N] = \
            corr[sl].transpose(2, 0, 1).reshape(E, BL * N)
        aux[E, AUX_C:AUX_C + BL * N] = np.tile(cnt, BL)
        in_maps.append({
            "ea": _to_ed(edge_attr[sl], variant),
            "aux": _to_ed(aux, variant),
            "mask": _to_ed(mask[sl].reshape(1, BL * N), variant),
            **consts,
        })
    return in_maps


def kernel(h, edge_attr, adj, num_nodes, W_w, W_b, U_w, U_b):
    variant = os.environ.get("KERNEL_VARIANT", "bf16")
    in_maps = prep_inputs(h, edge_attr, adj, num_nodes, W_w, W_b, U_w, U_b,
                          variant=variant)
    nc = build_nc(reps=1, variant=variant)
    res = run_bass_kernel_spmd(nc, in_maps, list(range(N_CORES)))
    out = np.empty((B, N, H), dtype=np.float32)
    for core in range(N_CORES):
        out[core * BL:(core + 1) * BL] = res.results[core]["y"]
    return out


# revision 20
# speedup vs baseline: 15.8833x; 3.2089x over previous
"""DMPNN layer kernel for Trainium2, data-parallel over batch on 8 NeuronCores.

Math (reference):
    gate[i,j]  = (sum_b adj[b,i,j]) > 0                      [N,N], shared across batch
    hW[b,i,o]  = sum_c h[b,i,c] * Wh[o,c]                    Wh = W_w[:, :H]
    term_h     = sum_i gate[i,j] * hW[b,i,o]
    e_sum      = sum_i gate[i,j] * edge_attr[b,i,j,e]
    term_e     = sum_e e_sum[b,j,e] * We[o,e]                We = W_w[:, H:]
    count[j]   = sum_i gate[i,j]
    msg        = term_h + term_e + count[j]*W_b[o]
    msg       *= (j < num_nodes[b])
    h_new      = (h + msg) @ U_w.T + U_b

Key restructuring vs a direct port:
  - The gated i-reduction of edge_attr is rewritten as
        e_sum = (sum over ALL i of ea) - corr,
    where corr[b,j,e] = sum_{i: gate[i,j]=0} ea[b,i,j,e] is computed EXACTLY
    on the host (it touches only the gate-complement entries, typically none:
    gate = any-over-32-random-batches is ~all-ones). This removes the
    elementwise gate Hadamard over the 16.8MB/core ea stream entirely —
    the device reduction becomes plain indicator-lhsT matmuls on PE.
  - All matmul operands are bf16 (PSUM accumulation stays fp32). This halves
    the dominant HBM stream. Host casts ea once.
  - Host pre-transposes h and the weight blocks; no device PE transposes.
  - term_h keeps the exact gate on device (tiny [N,N] matmul rhs).
  - The bias term count[j]*W_b[o] is folded into the correction matmul:
    corrT gets an extra row -cnt and the (negated) weight lhsT an extra
    column W_b, so  msg -= nwt17.T @ corrT17  adds both -We@corr and
    +W_b*cnt in one accumulating matmul.

DMA-ring scheduling (the real bottleneck): TRN2 has two HWDGE rings (SP and
ACT sequencers) plus the GpSimd SWDGE path; each `dma_start` occupies its
ring for the full transfer plus ~2us completion latency, serialized per
ring. So: ea batches (2MB each, the 8.4MB/core stream) alternate SP/ACT;
everything small is packed into ONE aux tensor [128, 2560] per rep on the
GpSimd ring, which also takes the y stores. PSUM->SBUF copies run on DVE so
the ACT engine is free to drive its DMA ring.
"""

import os
import sys

for _p in ("/opt/trn_rl_repo", "/root/.axon_site/_ro/trn_rl_repo"):
    if _p not in sys.path:
        sys.path.insert(0, _p)

import numpy as np

import concourse.bass as bass
import concourse.tile as tile
from concourse import bacc, mybir
from concourse.bass_utils import run_bass_kernel_spmd

B, N, H, E = 32, 256, 128, 16
N_CORES = 8
BL = B // N_CORES          # batches per core
NJE = N * E                # 4096
F32 = mybir.dt.float32
BF16 = mybir.dt.bfloat16
F32R = mybir.dt.float32r

# aux packing offsets (in ED elements, per partition)
AUX_GATE = 0               # [128, 2*N]   gate, (c j) packed
AUX_HT = 2 * N             # [128, BL*N]  hT, (b i) packed
AUX_C = AUX_HT + BL * N    # [18, BL*N]   corrT17 rows 0..16, mask row 17
AUX_W = AUX_C + BL * N     # total columns


def build_nc(reps: int = 1, variant: str = "bf16"):
    """variant: "bf16" - ea and all matmul operands bf16 (half DMA traffic)
                "f32r" - ea and matmul operands f32r (full-precision inputs)
                "nodma" - diagnostic: es matmuls read a preloaded const tile
                          (no ea stream)
                "dmaonly" - diagnostic: ea stream + trivial consumer only
                "nodep" - diagnostic: ea stream runs, es matmuls read const
                          (no data dependency between them)"""
    nodma = variant in ("nodma", "nodep")
    dmaonly = variant == "dmaonly"
    nodep = variant == "nodep"
    ED = F32R if variant == "f32r" else BF16

    nc = bacc.Bacc("TRN2", target_bir_lowering=False, debug=False,
                   num_devices=N_CORES)

    d_ea = nc.dram_tensor("ea", [BL, N, N, E], ED, kind="ExternalInput")
    d_aux = nc.dram_tensor("aux", [128, AUX_W], ED, kind="ExternalInput")
    d_mask = nc.dram_tensor("mask", [1, BL * N], ED, kind="ExternalInput")
    d_wht = nc.dram_tensor("wht", [H, H], ED, kind="ExternalInput")
    d_wet = nc.dram_tensor("wet", [E, H], ED, kind="ExternalInput")
    d_nwt = nc.dram_tensor("nwt", [E + 1, H], ED, kind="ExternalInput")
    d_uwt = nc.dram_tensor("uwt", [H, H], ED, kind="ExternalInput")
    d_ub2 = nc.dram_tensor("ub2", [1, 2 * H], F32, kind="ExternalInput")
    d_sel16 = nc.dram_tensor("sel16", [128, 256], ED, kind="ExternalInput")
    d_y = nc.dram_tensor("y", [BL, N, H], F32, kind="ExternalOutput")

    with tile.TileContext(nc) as tc:
        with (
            tc.tile_pool(name="const", bufs=1) as cpool,
            tc.tile_pool(name="perrep", bufs=2) as rpool,
            tc.tile_pool(name="ea", bufs=3) as eapool,
            tc.tile_pool(name="work", bufs=2) as wpool,
            tc.tile_pool(name="ps_es", bufs=2, space="PSUM") as ps_es,
            tc.tile_pool(name="ps_hw", bufs=2, space="PSUM") as ps_hw,
            tc.tile_pool(name="ps_msg", bufs=2, space="PSUM") as ps_msg,
            tc.tile_pool(name="ps_up", bufs=2, space="PSUM") as ps_up,
        ):
            # ---- constants (once per launch) ---------------------------
            wht = cpool.tile([H, H], ED)
            nc.sync.dma_start(wht[:], d_wht[:])
            wet = cpool.tile([E, H], ED)
            nc.sync.dma_start(wet[:], d_wet[:])
            nwt = cpool.tile([E + 1, H], ED)
            nc.sync.dma_start(nwt[:], d_nwt[:])
            uwt = cpool.tile([H, H], ED)
            nc.sync.dma_start(uwt[:], d_uwt[:])
            sel16 = cpool.tile([128, 256], ED)
            nc.sync.dma_start(sel16[:], d_sel16[:])
            ub_row = cpool.tile([1, 2 * H], F32)
            nc.sync.dma_start(ub_row[:], d_ub2[:])
            ub_b = cpool.tile([128, 2 * H], F32)
            nc.gpsimd.partition_broadcast(ub_b[:], ub_row[0:1, :])

            for rep in range(reps):
                # ---- per-rep shared loads (one SWDGE DMA) --------------
                aux = rpool.tile([128, AUX_W], ED, name="aux")
                nc.gpsimd.dma_start(aux[:], d_aux[:])
                gate_t = aux[:, AUX_GATE:AUX_GATE + 2 * N]
                ht_all = aux[:, AUX_HT:AUX_HT + BL * N]
                corrt = aux[0:E + 1, AUX_C:AUX_C + BL * N]
                maskb_all = rpool.tile([128, BL * N], ED, name="maskb")
                nc.gpsimd.dma_start(maskb_all[:],
                                    d_mask[0:1, :].partition_broadcast(128))

                hw = [None] * BL
                es = [None] * BL
                msg = [None] * BL

                def stage_front(b):
                    # hW natural [i, o], both i-chunks in one psum bank
                    hw_ps = ps_hw.tile([128, 2 * H], F32, name="hw_ps")
                    for c in range(2):
                        nc.tensor.matmul(
                            hw_ps[:, bass.ts(c, H)],
                            ht_all[:, b * N + c * 128:b * N + (c + 1) * 128],
                            wht[:], start=True, stop=True)
                    hw_sb = wpool.tile([128, 2 * H], ED, name="hw")
                    nc.vector.tensor_copy(hw_sb[:], hw_ps[:])
                    hw[b] = hw_sb

                    # ungated i-reduction of the ea stream -> esT [e, j]
                    if nodep:
                        ea_t = eapool.tile([128, 2 * NJE], ED, name="ea_t")
                        eng = nc.sync if b % 2 == 0 else nc.scalar
                        eng.dma_start(
                            ea_t[:].rearrange("p (c je) -> p c je", c=2),
                            d_ea[b, :, :, :].rearrange(
                                "(c p) j e -> p c (j e)", c=2))

                    if nodma and not nodep:
                        ea_t = aux  # any preloaded [128, >=2*NJE]... reuse
                    elif not nodep:
                        ea_t = eapool.tile([128, 2 * NJE], ED, name="ea_t")
                        eng = nc.sync if b % 2 == 0 else nc.scalar
                        eng.dma_start(
                            ea_t[:].rearrange("p (c je) -> p c je", c=2),
                            d_ea[b, :, :, :].rearrange(
                                "(c p) j e -> p c (j e)", c=2))
                    es_ps = ps_es.tile([E, N], F32, name="es_ps")
                    if nodma:
                        for c in range(2):
                            for e in range(E):
                                nc.tensor.matmul(
                                    es_ps[:, :], sel16[:, bass.ts(e, E)],
                                    aux[:, 0:N],
                                    start=(c == 0 and e == 0),
                                    stop=False)
                        # one matmul consumes the (otherwise-unused) ea DMA
                        nc.tensor.matmul(
                            es_ps[:], sel16[:, 0:E],
                            ea_t[:, 0:N] if nodep else aux[:, 0:N],
                            start=False, stop=True)
                    elif dmaonly:
                        ea_v = ea_t[:].rearrange("p (c j e) -> p c j e",
                                                 c=2, e=E)
                        nc.tensor.matmul(es_ps[:, :], sel16[:, 0:E],
                                         ea_v[:, 0, :, 0],
                                         start=True, stop=True)
                    else:
                        ea_v = ea_t[:].rearrange("p (c j e) -> p c j e",
                                                 c=2, e=E)
                        for c in range(2):
                            for e in range(E):
                                nc.tensor.matmul(es_ps[:, :],
                                                 sel16[:, bass.ts(e, E)],
                                                 ea_v[:, c, :, e],
                                                 start=(c == 0 and e == 0),
                                                 stop=(c == 1 and e == E - 1))
                    esr = wpool.tile([E, N], ED, name="esr")
                    nc.vector.tensor_copy(esr[:], es_ps[:])
                    es[b] = esr

                def stage_msg(b):
                    # msgT [o, j]: term_h (gate) + term_e - corr + bias
                    msg_ps = ps_msg.tile([H, N], F32, name="msg_ps")
                    for c in range(2):
                        nc.tensor.matmul(msg_ps[:], hw[b][:, bass.ts(c, H)],
                                         gate_t[:, bass.ts(c, N)],
                                         start=(c == 0), stop=False)
                    nc.tensor.matmul(msg_ps[:], wet[:], es[b][:],
                                     start=False, stop=False)
                    nc.tensor.matmul(msg_ps[:], nwt[:],
                                     corrt[:, bass.ts(b, N)],
                                     start=False, stop=True)
                    msg[b] = msg_ps

                def stage_back(b):
                    # xT = msgT*mask + hT ; y = xT_chunk.T @ uwT + ub
                    xT = wpool.tile([H, N], ED, name="xT")
                    nc.vector.tensor_tensor(xT[:], msg[b][:],
                                            maskb_all[:, bass.ts(b, N)],
                                            mybir.AluOpType.mult)
                    nc.vector.tensor_tensor(xT[:], xT[:],
                                            ht_all[:, bass.ts(b, N)],
                                            mybir.AluOpType.add)
                    up_ps = ps_up.tile([128, 2 * H], F32, name="up_ps")
                    for c in range(2):
                        nc.tensor.matmul(up_ps[:, bass.ts(c, H)],
                                         xT[:, bass.ts(c, 128)],
                                         uwt[:], start=True, stop=True)
                    yt = wpool.tile([128, 2 * H], F32, name="yt")
                    nc.vector.tensor_tensor(yt[:], up_ps[:], ub_b[:],
                                            mybir.AluOpType.add)
                    nc.gpsimd.dma_start(
                        d_y[b, :, :].rearrange("(c p) o -> p c o", c=2),
                        yt[:].rearrange("p (c o) -> p c o", c=2))

                for b in range(BL):
                    stage_front(b)
                    if b >= 1:
                        stage_msg(b - 1)
                    if b >= 2:
                        stage_back(b - 2)
                stage_msg(BL - 1)
                stage_back(BL - 2)
                stage_back(BL - 1)

    nc.compile()
    return nc


def _to_ed(a, variant):
    if variant == "f32r":
        return np.ascontiguousarray(a.astype(np.float32))
    import ml_dtypes
    return np.ascontiguousarray(a.astype(ml_dtypes.bfloat16))


def prep_inputs(h, edge_attr, adj, num_nodes, W_w, W_b, U_w, U_b,
                variant="bf16"):
    """Host-side sharding + restructuring. Returns per-core input maps."""
    h = np.asarray(h, dtype=np.float32)
    edge_attr = np.asarray(edge_attr, dtype=np.float32)
    adj = np.asarray(adj)
    nn = np.asarray(num_nodes).astype(np.int64)
    W_w = np.asarray(W_w, dtype=np.float32)
    W_b = np.asarray(W_b, dtype=np.float32)
    U_w = np.asarray(U_w, dtype=np.float32)
    U_b = np.asarray(U_b, dtype=np.float32)

    gate = (adj.sum(axis=0) > 0).astype(np.float32)          # [N, N]
    cnt = gate.sum(axis=0)                                   # [N]
    # exact gate-complement correction: corr[b,j,e] = sum_{i:gate=0} ea[b,i,j,e]
    zmask = gate == 0
    corr = np.zeros((B, N, E), dtype=np.float32)
    for j in np.flatnonzero(zmask.any(axis=0)):
        w = zmask[:, j].astype(np.float32)                   # [N] over i
        corr[:, j, :] = np.einsum('bie,i->be', edge_attr[:, :, j, :], w)

    mask = (np.arange(N)[None, :] < nn[:, None]).astype(np.float32)  # [B, N]
    Wh = W_w[:, :H]
    We = W_w[:, H:]
    sel16 = np.tile(np.eye(16, dtype=np.float32).reshape(1, 256), (128, 1))
    nwt17 = np.concatenate([-We.T, W_b.reshape(1, H)], axis=0)  # [17, H]

    consts = {
        "wht": _to_ed(Wh.T, variant),
        "wet": _to_ed(We.T, variant),
        "nwt": _to_ed(nwt17, variant),
        "uwt": _to_ed(U_w.T, variant),
        "ub2": np.ascontiguousarray(
            np.tile(U_b.reshape(1, H), (1, 2)).astype(np.float32)),
        "sel16": _to_ed(sel16, variant),
    }
    gate_pk = gate.reshape(2, 128, N).transpose(1, 0, 2).reshape(128, 2 * N)
    in_maps = []
    for core in range(N_CORES):
        sl = slice(core * BL, (core + 1) * BL)
        aux = np.zeros((128, AUX_W), dtype=np.float32)
        aux[:, AUX_GATE:AUX_GATE + 2 * N] = gate_pk
        # hT: aux[p, AUX_HT + b*N + i] = h[b, i, p]
        aux[:, AUX_HT:AUX_HT + BL * N] = \
            h[sl].transpose(2, 0, 1).reshape(H, BL * N)
        # corrT rows 0..15, bias row 16 = -cnt, mask row 17
        aux[0:E, AUX_C:AUX_C + BL * N] = \
            corr[sl].transpose(2, 0, 1).reshape(E, BL * N)
        aux[E, AUX_C:AUX_C + BL * N] = np.tile(cnt, BL)
        in_maps.append({
            "ea": _to_ed(edge_attr[sl], variant),
            "aux": _to_ed(aux, variant),
            "mask": _to_ed(mask[sl].reshape(1, BL * N), variant),
            **consts,
        })
    return in_maps


def kernel(h, edge_attr, adj, num_nodes, W_w, W_b, U_w, U_b):
    variant = os.environ.get("KERNEL_VARIANT", "bf16")
    in_maps = prep_inputs(h, edge_attr, adj, num_nodes, W_w, W_b, U_w, U_b,
                          variant=variant)
    nc = build_nc(reps=1, variant=variant)
    res = run_bass_kernel_spmd(nc, in_maps, list(range(N_CORES)))
    out = np.empty((B, N, H), dtype=np.float32)
    for core in range(N_CORES):
        out[core * BL:(core + 1) * BL] = res.results[core]["y"]
    return out
